# Initial kernel scaffold
#
"""Distributed causal multi-head attention for Trainium2 (8 NeuronCores).

Problem: B=2, S=2048, NX=1024, H=16 heads, D=64.
  qkv = x @ w_attn + b_attn ; q,k,v split; causal softmax(q k^T / 8) v ; @ w_proj + b_proj

Sharding: core c -> batch b=c//4 (data parallel), head group g=c%4 (tensor
parallel, 4 heads). Column-split c_attn; after attention four per-head
AllToAlls reshard heads->sequence so each core computes c_proj for its own
512 output rows with the full hidden dim - no cross-core reduction. The
batch-dependent A2A slot selection (write chunk qc to slot 4b+qc, read
senders 4b..4b+3) runs through a runtime register loaded from a per-core
input, so the c_proj contraction touches only the own-batch 1024 rows
(no junk-masked double contraction).

Layout strategy: host passes x transposed (xT [NX, S]) so QKV projections,
scores and PV products all run in matmul-native layouts with zero on-chip
transposes. Scores are computed transposed ([k, q]): the softmax reduction
over k lands on the partition axis, where an extra ones-column appended to V
yields the denominator for free in the same PV matmul. exp() needs no
max-subtraction (scores are bounded; ACT exp is <=2 ULP on [-10,10]).
Attention is key-block-major with up-to-1024-wide score tiles so the ScalarE
exp runs in few wide calls; the next head's first score tiles and earlier
heads' c_proj slices are emitted between exp and PV in the narrow tail
blocks so the PE never waits on exp. Matmul operands are bf16; accumulation
stays fp32 in PSUM. Each head's A2A fires as soon as that head finishes;
proj slices for heads 1-2 are held back to fill the last A2A's transfer
window, leaving only the last head's 2-matmul groups on the serial tail.
"""

import sys

sys.path.insert(0, "/opt/trn_rl_repo")

import numpy as np
import ml_dtypes

BF16 = ml_dtypes.bfloat16

B = 2
S = 2048
NX = 1024
H = 16
D = 64
G = 4            # head groups (tensor-parallel)
HL = H // G      # heads per core = 4
HDW = HL * D     # head-group width = 256
P = 128
SC = 512         # output chunk (A2A granularity)
NQC = S // SC    # 4 chunks
NE = NX // P     # 8 contraction tiles
NKB = S // P     # 16 key blocks
WQ = 1024        # max score-tile width
NPRE = 4         # next-head key blocks precomputed into this head's tail

_COMPILED = None


def _build():
    import concourse.bass as bass  # noqa: F401
    from concourse.bass import ds
    import concourse.mybir as mybir
    import concourse.tile as tile
    from concourse import bacc

    f32 = mybir.dt.float32
    i32 = mybir.dt.int32
    bf16 = mybir.dt.bfloat16
    Identity = mybir.ActivationFunctionType.Identity
    Exp = mybir.ActivationFunctionType.Exp

    nc = bacc.Bacc("TRN2", target_bir_lowering=False, debug=False, num_devices=8)

    x0 = nc.dram_tensor("x0", [NE, P, SC], bf16, kind="ExternalInput")
    xr = nc.dram_tensor("xr", [NE, P, S - SC], bf16, kind="ExternalInput")
    wqkf = nc.dram_tensor("wqkf", [4, P, NX], bf16, kind="ExternalInput")
    wvc = nc.dram_tensor("wvc", [P, NE * HDW], bf16, kind="ExternalInput")
    wpc = nc.dram_tensor("wpc", [P, 2 * HL * NX], bf16, kind="ExternalInput")
    bqk = nc.dram_tensor("bqk", [4, P], f32, kind="ExternalInput")
    bp32 = nc.dram_tensor("bp32", [1, NX], f32, kind="ExternalInput")
    onesb = nc.dram_tensor("onesb", [P, 4], bf16, kind="ExternalInput")
    causb = nc.dram_tensor("causb", [P, P], bf16, kind="ExternalInput")
    slotb = nc.dram_tensor("slotb", [1, 4], i32, kind="ExternalInput")
    out_ext = nc.dram_tensor("out", [SC, NX], f32, kind="ExternalOutput")

    with tile.TileContext(nc) as tc:
        with (
            tc.tile_pool(name="const", bufs=1) as const_pool,
            tc.tile_pool(name="xt", bufs=1) as xt_pool,
            tc.tile_pool(name="w", bufs=1) as w_pool,
            tc.tile_pool(name="qkt", bufs=1) as qkt_pool,
            tc.tile_pool(name="vsb", bufs=1) as v_pool,
            tc.tile_pool(name="lh", bufs=1) as lh_pool,
            tc.tile_pool(name="exp", bufs=12) as exp_pool,
            tc.tile_pool(name="osb", bufs=1) as osb_pool,
            tc.tile_pool(name="small", bufs=3) as small_pool,
            tc.tile_pool(name="wide", bufs=2, space="PSUM") as wide_ps,
            tc.tile_pool(name="atps", bufs=4, space="PSUM") as at_ps_pool,
            tc.tile_pool(name="dram", bufs=1, space="DRAM") as dram_pool,
        ):
            # ---- constants + the batch slot-base register (4*b) ----
            slot_sb = const_pool.tile([1, 4], i32, name="slot_sb")
            nc.sync.dma_start(slot_sb[:], slotb[:])
            sreg = nc.sync.alloc_register("slotreg")
            nc.sync.reg_load(sreg, slot_sb[0:1, 0:1])
            rv = nc.sync.snap(sreg, donate=True, min_val=0, max_val=4)

            bqk_sb = const_pool.tile([P, 4], f32, name="bqk_sb")
            for fi in range(4):
                nc.sync.dma_start(bqk_sb[:, fi : fi + 1], bqk[fi : fi + 1, :])
            bp_sb = const_pool.tile([1, NX], f32, name="bp_sb")
            nc.sync.dma_start(bp_sb[:], bp32[:])
            czb_sb = const_pool.tile([P, P], bf16, name="czb_sb")
            nc.sync.dma_start(czb_sb[:], causb[:])

            # ---- weight + x loads, big-line DMAs, first-needed first ----
            wqk_sb = {}
            for fi in range(4):
                wqk_sb[fi] = w_pool.tile([P, NX], bf16, name=f"wqk_sb{fi}")
            nc.sync.dma_start(wqk_sb[0][:], wqkf[0])
            xt0_sb = {}
            wv_sb = {}
            for e in range(NE):
                t = xt_pool.tile([P, SC], bf16, name=f"xt0_{e}")
                nc.sync.dma_start(t[:], x0[e])
                xt0_sb[e] = t
                # per-e v-weight tiles interleaved with the x loads: the
                # in-order PE's first v matmuls then wait only on their own
                # 64KB slice instead of one late monolithic load
                tv = w_pool.tile([P, HDW], bf16, name=f"wv_sb{e}")
                nc.sync.dma_start(tv[:], wvc[:, e * HDW : (e + 1) * HDW])
                wv_sb[e] = tv
            for fi in range(1, 4):
                nc.sync.dma_start(wqk_sb[fi][:], wqkf[fi])
            xtr_sb = {}
            for e in range(NE):
                t = xt_pool.tile([P, S - SC], bf16, name=f"xtr_{e}")
                nc.sync.dma_start(t[:], xr[e])
                xtr_sb[e] = t
            wp_sb = w_pool.tile([P, 2 * HL * NX], bf16, name="wp_sb")
            nc.sync.dma_start(wp_sb[:], wpc[:])

            def xt_slice(e, c0, w):
                # columns [c0, c0+w) of the token axis for contraction tile e
                if c0 < SC:
                    return xt0_sb[e][:, c0 : c0 + w]
                return xtr_sb[e][:, c0 - SC : c0 - SC + w]

            # ---- phase 1: qkT [2*HDW, S] (full-S tiles) and v [S, padded] ----
            qkt_sb = {}
            v_sb = {}
            for fi in range(2):
                qkt_sb[fi] = qkt_pool.tile(
                    [P, S], bf16, name=f"qkt{fi}", tag=f"qktw{fi}"
                )
            # per-head kT with the other head's rows zeroed: score matmuls
            # then run at K=128 (zeros annihilate the foreign q rows), which
            # keeps switching activity low for the HAM power governor
            ktz_sb = {}
            for h in range(HL):
                ktz_sb[h] = qkt_pool.tile([P, S], bf16, name=f"ktz{h}", tag=f"ktz{h}")
                nc.gpsimd.memset(ktz_sb[h][:], 0.0)
            for sc in range(NQC):
                # interleave each qk chain with a v chain: adjacent matmuls
                # hit different PSUM banks with independent stationaries, so
                # the PE pipelines them (same-chain neighbors serialize on
                # the accumulation bank)
                for j in range(4):
                    fi = j
                    si = 4 * sc + j
                    ps = at_ps_pool.tile([P, SC], f32, tag="atps", name=f"qk_ps{fi}_{sc}")
                    psv = wide_ps.tile([P, HDW], f32, tag="wide", name=f"v_ps{si}")
                    for e in range(NE):
                        nc.tensor.matmul(
                            ps[:],
                            wqk_sb[fi][:, e * P : (e + 1) * P],
                            xt_slice(e, sc * SC, SC),
                            start=(e == 0),
                            stop=(e == NE - 1),
                        )
                        nc.tensor.matmul(
                            psv[:],
                            xt_slice(e, sc * SC + j * P, P),
                            wv_sb[e][:],
                            start=(e == 0),
                            stop=(e == NE - 1),
                        )
                    # fold the 1/sqrt(D)=1/8 score scale into q (bias comes
                    # pre-scaled from the host)
                    if fi < 2:
                        nc.scalar.activation(
                            qkt_sb[fi][:, sc * SC : (sc + 1) * SC],
                            ps[:],
                            Identity,
                            bias=bqk_sb[:, fi : fi + 1],
                            scale=0.125,
                        )
                    else:
                        for hh in range(2):
                            h = 2 * (fi - 2) + hh
                            r0 = 64 * hh
                            nc.scalar.activation(
                                ktz_sb[h][r0 : r0 + D, sc * SC : (sc + 1) * SC],
                                ps[r0 : r0 + D, :],
                                Identity,
                                bias=bqk_sb[r0 : r0 + D, fi : fi + 1],
                            )
                    # per-head 128-wide slots: [v(64) | ones(1) | zeros(63)]
                    vt = v_pool.tile([P, HL * P], bf16, name=f"v{si}")
                    nc.gpsimd.memset(vt[:], 0.0)
                    nc.sync.dma_start(
                        vt[:].rearrange("p (h u) -> p h u", h=HL)[:, :, D : D + 1],
                        onesb[:],
                    )
                    nc.scalar.activation(
                        vt[:].rearrange("p (h u) -> p h u", h=HL)[:, :, 0:D],
                        psv[:].rearrange("p (h u) -> p h u", h=HL),
                        Identity,
                    )
                    v_sb[si] = vt

            # ---- phase 2: attention, key-block-major per head ----
            # one A2A per head h: chunk qc of head h -> slots qc and qc+4
            # (both batches; static DMA descriptors stay on the fast hardware
            # descgen path). The receiver picks its own-batch slots 4b..4b+3
            # with ONE dynamic contiguous DRAM->DRAM copy into lhsrc (a single
            # software-generated descriptor), so the c_proj contraction is
            # junk-free without per-line software DMA descgen.
            a2a_in = {}
            a2a_out = {}
            lhsrc = {}
            for h in range(HL):
                a2a_in[h] = dram_pool.tile([8, D, SC], bf16, name=f"a2a_in{h}")
                a2a_out[h] = dram_pool.tile([8, D, SC], bf16, name=f"a2a_out{h}")
                lhsrc[h] = dram_pool.tile([4, D, SC], bf16, name=f"lhsrc{h}")

            def emit_tail(h, qc, at_ps):
                # (the v-bias is folded into bp on the host: softmax rows sum
                # to 1, so P@(v + 1 bv^T) @ wp = P@v@wp + bv@wp)
                dn32 = small_pool.tile([1, SC], f32, tag="dn32", name=f"dn32{qc}_{h}")
                nc.vector.tensor_copy(dn32[:], at_ps[D : D + 1, :])
                rc32 = small_pool.tile([1, SC], f32, tag="rc32", name=f"rc32{qc}_{h}")
                nc.vector.reciprocal_approx_fast(rc32[:], dn32[:])
                # broadcast 1/denom across partitions on the (idle) GpSimd
                rb = small_pool.tile([D, SC], f32, tag="rbsb", name=f"rbsb{qc}_{h}")
                nc.gpsimd.partition_broadcast(rb[:], rc32[:])
                ath = small_pool.tile(
                    [D, SC], bf16, tag="ath", bufs=3, name=f"ath{qc}_{h}"
                )
                nc.vector.tensor_mul(ath[:], at_ps[0:D, :], rb[:])
                nc.sync.dma_start(a2a_in[h][qc, :, :], ath[:])
                nc.sync.dma_start(a2a_in[h][qc + 4, :, :], ath[:])

            def emit_score_exp(h, kb):
                # score tiles (transposed [k, q]) + exp for key block kb of
                # head h; returns the bf16 exp tiles with their column spans
                fi_q = h // 2
                q0 = P * kb
                width = S - q0
                out = []
                for s2 in range((width + WQ - 1) // WQ):
                    w0 = q0 + s2 * WQ
                    ww = min(WQ, S - w0)
                    scp = wide_ps.tile(
                        [P, WQ], f32, tag="wide", name=f"sc{h}_{kb}_{s2}"
                    )
                    # scoresT[k, q] = kT^T qT (contraction over d, zero-
                    # padded to K=128); matmul output is capped at one
                    # PSUM bank (512 f32)
                    for m0 in range(0, ww, SC):
                        mw = min(SC, ww - m0)
                        nc.tensor.matmul(
                            scp[:, m0 : m0 + mw],
                            ktz_sb[h][:, q0 : q0 + P],
                            qkt_sb[fi_q][:, w0 + m0 : w0 + m0 + mw],
                            start=True,
                            stop=True,
                        )
                    ex = exp_pool.tile(
                        [P, WQ], bf16, tag="exp", name=f"ex{h}_{kb}_{s2}"
                    )
                    nc.scalar.activation(ex[:, 0:ww], scp[:, 0:ww], Exp)
                    if s2 == 0:
                        # diagonal block: zero the non-causal upper triangle
                        # of the exp weights (0/1 mask on DVE; saves a PE
                        # matmul per key block — each costs ~200ns fixed)
                        nc.vector.tensor_mul(ex[:, 0:P], ex[:, 0:P], czb_sb[:])
                    out.append((ex, w0, ww))
                return out

            def emit_pv(h, kb, exl, at_ps, qcs=range(NQC)):
                # PV pieces per output chunk qc (+denominator via the
                # ones column of v)
                for ex, w0, ww in exl:
                    qc_lo = w0 // SC
                    qc_hi = (w0 + ww - 1) // SC
                    for qc in range(qc_lo, qc_hi + 1):
                        if qc not in qcs:
                            continue
                        a0 = max(w0, qc * SC)
                        a1 = min(w0 + ww, (qc + 1) * SC)
                        nc.tensor.matmul(
                            at_ps[qc][:, a0 - qc * SC : a1 - qc * SC],
                            v_sb[kb][:, h * P : (h + 1) * P],
                            ex[:, a0 - w0 : a1 - w0],
                            start=(kb == 0),
                            stop=(kb == 4 * qc + 3),
                        )

            # ---- phase 3 pieces: own-batch gather + c_proj slices, emitted
            # interleaved into later heads' attention (PE runs in program
            # order). Contraction tile (h, tp): partitions 0:64 = in-batch
            # sender 2tp's head-h rows, 64:128 = sender 2tp+1's.
            osb = {}
            for st in range(4):
                osb[st] = osb_pool.tile([P, NX], f32, name=f"osb{st}")
                # b_proj preloaded across all partitions (idle GpSimd);
                # every proj group is then a plain accumulate
                nc.gpsimd.partition_broadcast(osb[st][:], bp_sb[:])
            lh = {}

            def emit_lh_loads(h):
                for tp in range(2):
                    t = lh_pool.tile([P, SC], bf16, name=f"lh{h}_{tp}")
                    # two half-tile loads on purpose: they land on different
                    # DMA queues and run in parallel (one fused [128,512]
                    # load serializes on a single engine and measures slower)
                    nc.sync.dma_start(t[0:D, :], lhsrc[h][2 * tp])
                    nc.sync.dma_start(t[D:P, :], lhsrc[h][2 * tp + 1])
                    lh[h, tp] = t

            def emit_proj_group(h, st, nn2):
                pp = wide_ps.tile([P, SC], f32, tag="wide", name=f"pj{h}_{st}_{nn2}")
                for tp in range(2):
                    wcol = (2 * h + tp) * NX + nn2 * SC
                    nc.tensor.matmul(
                        pp[:],
                        lh[h, tp][:, st * P : (st + 1) * P],
                        wp_sb[:, wcol : wcol + SC],
                        start=(tp == 0),
                        stop=(tp == 1),
                    )
                dst = osb[st][:, nn2 * SC : (nn2 + 1) * SC]
                nc.vector.tensor_add(dst, dst, pp[:])
                if h == HL - 1 and nn2 == 1:
                    nc.sync.dma_start(out_ext[st * P : (st + 1) * P, :], osb[st][:])

            # ---- dummy A2A fired at the end of phase 1: resyncs the cores'
            # launch/throttle drift right before the real A2As start, and
            # absorbs the collective stream's one-time setup cost ----
            a2ad_in = dram_pool.tile([8, 1, P], bf16, name="a2ad_in")
            a2ad_out = dram_pool.tile([8, 1, P], bf16, name="a2ad_out")
            for s in range(8):
                nc.sync.dma_start(a2ad_in[s], v_sb[15][0:1, 0:P])
            nc.gpsimd.collective_compute(
                "AllToAll",
                mybir.AluOpType.bypass,
                ins=[a2ad_in[:].opt()],
                outs=[a2ad_out[:].opt()],
                replica_groups=[list(range(8))],
            )

            # proj interleave schedule: h0's groups during h2 (late blocks),
            # 4 of h1's groups into h3's tail blocks; h1's other 4 + all of
            # h2's stay after h3 to fill the last A2A's transfer window
            interleave = {2: {}, 3: {}}
            islots = [11, 13, 14, 15]
            for i in range(4):
                interleave[2][islots[i]] = (0, i // 2, i % 2)
            for i in range(4):
                interleave[3][12 + i] = (1, i // 2, i % 2)

            saved = {}
            for h in range(HL):
                at_ps = {}
                done = set()
                for qc in range(NQC):
                    at_ps[qc] = at_ps_pool.tile(
                        [P, SC], f32, tag="atps", name=f"at_ps{qc}_{h}"
                    )
                if h >= 2:
                    emit_lh_loads(h - 2)
                for kb in range(NKB):
                    if (h, kb) in saved:
                        exl = saved.pop((h, kb))
                    else:
                        exl = emit_score_exp(h, kb)
                    # fills between exp and the PV that consumes it: next
                    # head's first (widest) score tiles and earlier heads'
                    # proj groups keep the PE busy while ScalarE runs exp
                    if h + 1 < HL and kb >= NKB - NPRE:
                        kpre = kb - (NKB - NPRE)
                        saved[h + 1, kpre] = emit_score_exp(h + 1, kpre)
                    pj = interleave.get(h, {}).get(kb)
                    if pj is not None:
                        emit_proj_group(*pj)
                    emit_pv(h, kb, exl, at_ps)
                    # immediate normalization: the tail chain is fully
                    # DVE/GpSimd now (no PE broadcast matmul), so deferring
                    # would only delay accumulator frees and A2A staging
                    defer = 0
                    for qc in range(NQC):
                        if kb == min(4 * qc + 3 + defer, NKB - 1) and qc not in done:
                            done.add(qc)
                            emit_tail(h, qc, at_ps[qc])
                nc.gpsimd.collective_compute(
                    "AllToAll",
                    mybir.AluOpType.bypass,
                    ins=[a2a_in[h][:].opt()],
                    outs=[a2a_out[h][:].opt()],
                    replica_groups=[list(range(8))],
                )
                # own-batch slot select: one contiguous 256KB dynamic copy
                nc.sync.dma_start(lhsrc[h][:], a2a_out[h][ds(rv, 4)])

            # tail: h0's and h1's remaining + h2's proj groups overlap the
            # h2/h3 A2A transfers; then the last head's own slice
            for i in range(4):
                emit_proj_group(0, 2 + i // 2, i % 2)
            for i in range(4):
                emit_proj_group(1, 2 + i // 2, i % 2)
            emit_lh_loads(2)
            for st in range(4):
                for nn2 in range(2):
                    emit_proj_group(2, st, nn2)
            emit_lh_loads(3)
            for st in range(4):
                for nn2 in range(2):
                    emit_proj_group(3, st, nn2)

    nc.compile()
    return nc


def _get_compiled():
    global _COMPILED
    if _COMPILED is None:
        _COMPILED = _build()
    return _COMPILED


def make_in_maps(x, attention_mask, w_attn, b_attn, w_proj, b_proj):
    x = np.asarray(x, dtype=np.float32)
    w_attn = np.asarray(w_attn, dtype=np.float32)
    b_attn = np.asarray(b_attn, dtype=np.float32)
    w_proj = np.asarray(w_proj, dtype=np.float32)
    b_proj = np.asarray(b_proj, dtype=np.float32)

    ki, qi = np.meshgrid(np.arange(P), np.arange(P), indexing="ij")
    causalT = np.where(ki > qi, np.float32(0.0), np.float32(1.0))
    # xT [NX, S] -> e-major [NE, P, S], split [:, :, :SC] / [:, :, SC:]
    x8 = [
        np.ascontiguousarray(x[b].T.astype(BF16).reshape(NE, P, S)) for b in range(B)
    ]
    bv_full = b_attn[2 * NX : 3 * NX].astype(np.float64)
    bp_eff = (b_proj.astype(np.float64) + bv_full @ w_proj.astype(np.float64)).astype(
        np.float32
    )
    bp_row32 = np.ascontiguousarray(bp_eff.reshape(1, NX))

    in_maps = []
    for c in range(8):
        b, g = divmod(c, 4)
        cols = slice(HDW * g, HDW * (g + 1))
        kcols = slice(NX + HDW * g, NX + HDW * (g + 1))
        vcols = slice(2 * NX + HDW * g, 2 * NX + HDW * (g + 1))
        bqk_arr = np.concatenate([b_attn[cols] * 0.125, b_attn[kcols]]).reshape(4, P)
        # fi-major q/k weights: wqkf[fi] = [P, NX] with stationary (fi, e)
        # at [:, e*128:(e+1)*128]
        wqk = np.concatenate([w_attn[:, cols], w_attn[:, kcols]], axis=1)  # [NX, 512]
        wqkf = np.ascontiguousarray(
            wqk.reshape(NE, P, 4, P).transpose(2, 1, 0, 3).reshape(4, P, NX)
        ).astype(BF16)
        wvc = np.ascontiguousarray(
            w_attn[:, vcols].reshape(NE, P, HDW).transpose(1, 0, 2).reshape(P, NE * HDW)
        ).astype(BF16)
        # own-batch proj tiles (h, tp): rows 0:64 = in-batch sender 2tp's
        # head-h w_proj rows, 64:128 = sender 2tp+1's
        wtiles = np.zeros((HL, 2, P, NX), dtype=np.float32)
        for h in range(HL):
            for tp in range(2):
                for half, j in ((0, 2 * tp), (1, 2 * tp + 1)):
                    rows = w_proj[HDW * j + D * h : HDW * j + D * (h + 1), :]
                    wtiles[h, tp, 64 * half : 64 * half + D, :] = rows
        wpc = np.ascontiguousarray(
            wtiles.reshape(2 * HL, P, NX).transpose(1, 0, 2).reshape(P, 2 * HL * NX)
        ).astype(BF16)
        in_maps.append(
            {
                "x0": np.ascontiguousarray(x8[b][:, :, :SC]),
                "xr": np.ascontiguousarray(x8[b][:, :, SC:]),
                "wqkf": wqkf,
                "wvc": wvc,
                "wpc": wpc,
                "bqk": np.ascontiguousarray(bqk_arr),
                "bp32": bp_row32,
                "causb": causalT.astype(BF16),
                "onesb": np.ones((P, 4), dtype=BF16),
                "slotb": np.array([[4 * b, 0, 0, 0]], dtype=np.int32),
            }
        )
    return in_maps


def assemble_out(results):
    out = np.empty((B, S, NX), dtype=np.float32)
    for c in range(8):
        b, g = divmod(c, 4)
        out[b, g * SC : (g + 1) * SC, :] = results[c]["out"]
    return out


def run(in_maps, trace=False):
    from concourse.bass_utils import run_bass_kernel_spmd

    nc = _get_compiled()
    return run_bass_kernel_spmd(nc, in_maps, core_ids=list(range(8)), trace=trace)


def kernel(**inputs) -> np.ndarray:
    in_maps = make_in_maps(**inputs)
    res = run(in_maps)
    return assemble_out(res.results)


if __name__ == "__main__":
    _get_compiled()
    print("build+compile OK")



# revision 1
# speedup vs baseline: 1.2444x; 1.2444x over previous
"""Distributed causal multi-head attention for Trainium2 (8 NeuronCores).

Problem: B=2, S=2048, NX=1024, H=16 heads, D=64.
  qkv = x @ w_attn + b_attn ; q,k,v split; causal softmax(q k^T / 8) v ; @ w_proj + b_proj

Sharding: core c -> batch b=c//4 (data parallel), head group g=c%4 (tensor
parallel, 4 heads). Column-split c_attn; after attention four per-head
AllToAlls reshard heads->sequence so each core computes c_proj for its own
512 output rows with the full hidden dim - no cross-core reduction. The
batch-dependent A2A slot selection (write chunk qc to slot 4b+qc, read
senders 4b..4b+3) runs through a runtime register loaded from a per-core
input, so the c_proj contraction touches only the own-batch 1024 rows
(no junk-masked double contraction).

Layout strategy: host passes x transposed (xT [NX, S]) so QKV projections,
scores and PV products all run in matmul-native layouts with zero on-chip
transposes. Scores are computed transposed ([k, q]): the softmax reduction
over k lands on the partition axis, where an extra ones-column appended to V
yields the denominator for free in the same PV matmul. exp() needs no
max-subtraction (scores are bounded; ACT exp is <=2 ULP on [-10,10]).
Attention is key-block-major with up-to-1024-wide score tiles so the ScalarE
exp runs in few wide calls; the next head's first score tiles and earlier
heads' c_proj slices are emitted between exp and PV in the narrow tail
blocks so the PE never waits on exp. Matmul operands are bf16; accumulation
stays fp32 in PSUM. Each head's A2A fires as soon as that head finishes;
proj slices for heads 1-2 are held back to fill the last A2A's transfer
window, leaving only the last head's 2-matmul groups on the serial tail.
"""

import sys

sys.path.insert(0, "/opt/trn_rl_repo")

import numpy as np
import ml_dtypes

BF16 = ml_dtypes.bfloat16

B = 2
S = 2048
NX = 1024
H = 16
D = 64
G = 4            # head groups (tensor-parallel)
HL = H // G      # heads per core = 4
HDW = HL * D     # head-group width = 256
P = 128
SC = 512         # output chunk (A2A granularity)
NQC = S // SC    # 4 chunks
NE = NX // P     # 8 contraction tiles
NKB = S // P     # 16 key blocks
WQ = 1024        # max score-tile width
NPRE = 4         # next-head key blocks precomputed into this head's tail

_COMPILED = None


def _build():
    import concourse.bass as bass  # noqa: F401
    from concourse.bass import ds
    import concourse.mybir as mybir
    import concourse.tile as tile
    from concourse import bacc

    f32 = mybir.dt.float32
    i32 = mybir.dt.int32
    bf16 = mybir.dt.bfloat16
    Identity = mybir.ActivationFunctionType.Identity
    Exp = mybir.ActivationFunctionType.Exp

    nc = bacc.Bacc("TRN2", target_bir_lowering=False, debug=False, num_devices=8)

    x0 = nc.dram_tensor("x0", [NE, P, SC], bf16, kind="ExternalInput")
    xr = nc.dram_tensor("xr", [NE, P, S - SC], bf16, kind="ExternalInput")
    wqkf = nc.dram_tensor("wqkf", [4, P, NX], bf16, kind="ExternalInput")
    wvc = nc.dram_tensor("wvc", [P, NE * HDW], bf16, kind="ExternalInput")
    wpc = nc.dram_tensor("wpc", [P, 2 * HL * NX], bf16, kind="ExternalInput")
    bqk = nc.dram_tensor("bqk", [4, P], f32, kind="ExternalInput")
    bp32 = nc.dram_tensor("bp32", [1, NX], f32, kind="ExternalInput")
    onesb = nc.dram_tensor("onesb", [P, 4], bf16, kind="ExternalInput")
    causb = nc.dram_tensor("causb", [P, P], bf16, kind="ExternalInput")
    slotb = nc.dram_tensor("slotb", [1, 4], i32, kind="ExternalInput")
    out_ext = nc.dram_tensor("out", [SC, NX], f32, kind="ExternalOutput")

    with tile.TileContext(nc) as tc:
        with (
            tc.tile_pool(name="const", bufs=1) as const_pool,
            tc.tile_pool(name="xt", bufs=1) as xt_pool,
            tc.tile_pool(name="w", bufs=1) as w_pool,
            tc.tile_pool(name="qkt", bufs=1) as qkt_pool,
            tc.tile_pool(name="vsb", bufs=1) as v_pool,
            tc.tile_pool(name="lh", bufs=1) as lh_pool,
            tc.tile_pool(name="exp", bufs=12) as exp_pool,
            tc.tile_pool(name="osb", bufs=1) as osb_pool,
            tc.tile_pool(name="small", bufs=3) as small_pool,
            tc.tile_pool(name="wide", bufs=2, space="PSUM") as wide_ps,
            tc.tile_pool(name="atps", bufs=4, space="PSUM") as at_ps_pool,
            tc.tile_pool(name="dram", bufs=1, space="DRAM") as dram_pool,
        ):
            # ---- constants + the batch slot-base register (4*b) ----
            slot_sb = const_pool.tile([1, 4], i32, name="slot_sb")
            nc.sync.dma_start(slot_sb[:], slotb[:])
            sreg = nc.sync.alloc_register("slotreg")
            nc.sync.reg_load(sreg, slot_sb[0:1, 0:1])
            rv = nc.sync.snap(sreg, donate=True, min_val=0, max_val=4)

            bqk_sb = const_pool.tile([P, 4], f32, name="bqk_sb")
            for fi in range(4):
                nc.sync.dma_start(bqk_sb[:, fi : fi + 1], bqk[fi : fi + 1, :])
            bp_sb = const_pool.tile([1, NX], f32, name="bp_sb")
            nc.sync.dma_start(bp_sb[:], bp32[:])
            czb_sb = const_pool.tile([P, P], bf16, name="czb_sb")
            nc.sync.dma_start(czb_sb[:], causb[:])

            # ---- weight + x loads, big-line DMAs, first-needed first ----
            wqk_sb = {}
            for fi in range(4):
                wqk_sb[fi] = w_pool.tile([P, NX], bf16, name=f"wqk_sb{fi}")
            nc.sync.dma_start(wqk_sb[0][:], wqkf[0])
            xt0_sb = {}
            wv_sb = {}
            for e in range(NE):
                t = xt_pool.tile([P, SC], bf16, name=f"xt0_{e}")
                nc.sync.dma_start(t[:], x0[e])
                xt0_sb[e] = t
                # per-e v-weight tiles interleaved with the x loads: the
                # in-order PE's first v matmuls then wait only on their own
                # 64KB slice instead of one late monolithic load
                tv = w_pool.tile([P, HDW], bf16, name=f"wv_sb{e}")
                nc.sync.dma_start(tv[:], wvc[:, e * HDW : (e + 1) * HDW])
                wv_sb[e] = tv
            for fi in range(1, 4):
                nc.sync.dma_start(wqk_sb[fi][:], wqkf[fi])
            xtr_sb = {}
            for e in range(NE):
                t = xt_pool.tile([P, S - SC], bf16, name=f"xtr_{e}")
                nc.sync.dma_start(t[:], xr[e])
                xtr_sb[e] = t
            wp_sb = w_pool.tile([P, 2 * HL * NX], bf16, name="wp_sb")
            nc.sync.dma_start(wp_sb[:], wpc[:])

            def xt_slice(e, c0, w):
                # columns [c0, c0+w) of the token axis for contraction tile e
                if c0 < SC:
                    return xt0_sb[e][:, c0 : c0 + w]
                return xtr_sb[e][:, c0 - SC : c0 - SC + w]

            # ---- phase 1: qkT [2*HDW, S] (full-S tiles) and v [S, padded] ----
            qkt_sb = {}
            v_sb = {}
            for fi in range(2):
                qkt_sb[fi] = qkt_pool.tile(
                    [P, S], bf16, name=f"qkt{fi}", tag=f"qktw{fi}"
                )
            # per-head kT with the other head's rows zeroed: score matmuls
            # then run at K=128 (zeros annihilate the foreign q rows), which
            # keeps switching activity low for the HAM power governor
            ktz_sb = {}
            for h in range(HL):
                ktz_sb[h] = qkt_pool.tile([P, S], bf16, name=f"ktz{h}", tag=f"ktz{h}")
                nc.gpsimd.memset(ktz_sb[h][:], 0.0)
            for sc in range(NQC):
                # interleave each qk chain with a v chain: adjacent matmuls
                # hit different PSUM banks with independent stationaries, so
                # the PE pipelines them (same-chain neighbors serialize on
                # the accumulation bank)
                for j in range(4):
                    fi = j
                    si = 4 * sc + j
                    ps = at_ps_pool.tile([P, SC], f32, tag="atps", name=f"qk_ps{fi}_{sc}")
                    psv = wide_ps.tile([P, HDW], f32, tag="wide", name=f"v_ps{si}")
                    for e in range(NE):
                        nc.tensor.matmul(
                            ps[:],
                            wqk_sb[fi][:, e * P : (e + 1) * P],
                            xt_slice(e, sc * SC, SC),
                            start=(e == 0),
                            stop=(e == NE - 1),
                        )
                        nc.tensor.matmul(
                            psv[:],
                            xt_slice(e, sc * SC + j * P, P),
                            wv_sb[e][:],
                            start=(e == 0),
                            stop=(e == NE - 1),
                        )
                    # fold the 1/sqrt(D)=1/8 score scale into q (bias comes
                    # pre-scaled from the host)
                    if fi < 2:
                        nc.scalar.activation(
                            qkt_sb[fi][:, sc * SC : (sc + 1) * SC],
                            ps[:],
                            Identity,
                            bias=bqk_sb[:, fi : fi + 1],
                            scale=0.125,
                        )
                    else:
                        for hh in range(2):
                            h = 2 * (fi - 2) + hh
                            r0 = 64 * hh
                            nc.scalar.activation(
                                ktz_sb[h][r0 : r0 + D, sc * SC : (sc + 1) * SC],
                                ps[r0 : r0 + D, :],
                                Identity,
                                bias=bqk_sb[r0 : r0 + D, fi : fi + 1],
                            )
                    # per-head 128-wide slots: [v(64) | ones(1) | zeros(63)]
                    vt = v_pool.tile([P, HL * P], bf16, name=f"v{si}")
                    nc.gpsimd.memset(vt[:], 0.0)
                    nc.sync.dma_start(
                        vt[:].rearrange("p (h u) -> p h u", h=HL)[:, :, D : D + 1],
                        onesb[:],
                    )
                    nc.scalar.activation(
                        vt[:].rearrange("p (h u) -> p h u", h=HL)[:, :, 0:D],
                        psv[:].rearrange("p (h u) -> p h u", h=HL),
                        Identity,
                    )
                    v_sb[si] = vt

            # ---- phase 2: attention, key-block-major per head ----
            # one A2A per head h: chunk qc of head h -> slots qc and qc+4
            # (both batches; static DMA descriptors stay on the fast hardware
            # descgen path). The receiver picks its own-batch slots 4b..4b+3
            # with ONE dynamic contiguous DRAM->DRAM copy into lhsrc (a single
            # software-generated descriptor), so the c_proj contraction is
            # junk-free without per-line software DMA descgen.
            a2a_in = {}
            a2a_out = {}
            lhsrc = {}
            for h in range(HL):
                a2a_in[h] = dram_pool.tile([8, D, SC], bf16, name=f"a2a_in{h}")
                a2a_out[h] = dram_pool.tile([8, D, SC], bf16, name=f"a2a_out{h}")
                lhsrc[h] = dram_pool.tile([4, D, SC], bf16, name=f"lhsrc{h}")

            def emit_tail(h, qc, at_ps):
                # (the v-bias is folded into bp on the host: softmax rows sum
                # to 1, so P@(v + 1 bv^T) @ wp = P@v@wp + bv@wp)
                dn32 = small_pool.tile([1, SC], f32, tag="dn32", name=f"dn32{qc}_{h}")
                nc.vector.tensor_copy(dn32[:], at_ps[D : D + 1, :])
                rc32 = small_pool.tile([1, SC], f32, tag="rc32", name=f"rc32{qc}_{h}")
                nc.vector.reciprocal_approx_fast(rc32[:], dn32[:])
                # broadcast 1/denom across partitions on the (idle) GpSimd
                rb = small_pool.tile([D, SC], f32, tag="rbsb", name=f"rbsb{qc}_{h}")
                nc.gpsimd.partition_broadcast(rb[:], rc32[:])
                ath = small_pool.tile(
                    [D, SC], bf16, tag="ath", bufs=3, name=f"ath{qc}_{h}"
                )
                nc.vector.tensor_mul(ath[:], at_ps[0:D, :], rb[:])
                nc.sync.dma_start(a2a_in[h][qc, :, :], ath[:])
                nc.sync.dma_start(a2a_in[h][qc + 4, :, :], ath[:])

            def emit_score_exp(h, kb):
                # score tiles (transposed [k, q]) + exp for key block kb of
                # head h; returns the bf16 exp tiles with their column spans
                fi_q = h // 2
                q0 = P * kb
                width = S - q0
                out = []
                for s2 in range((width + WQ - 1) // WQ):
                    w0 = q0 + s2 * WQ
                    ww = min(WQ, S - w0)
                    scp = wide_ps.tile(
                        [P, WQ], f32, tag="wide", name=f"sc{h}_{kb}_{s2}"
                    )
                    # scoresT[k, q] = kT^T qT (contraction over d, zero-
                    # padded to K=128); matmul output is capped at one
                    # PSUM bank (512 f32)
                    for m0 in range(0, ww, SC):
                        mw = min(SC, ww - m0)
                        nc.tensor.matmul(
                            scp[:, m0 : m0 + mw],
                            ktz_sb[h][:, q0 : q0 + P],
                            qkt_sb[fi_q][:, w0 + m0 : w0 + m0 + mw],
                            start=True,
                            stop=True,
                        )
                    ex = exp_pool.tile(
                        [P, WQ], bf16, tag="exp", name=f"ex{h}_{kb}_{s2}"
                    )
                    nc.scalar.activation(ex[:, 0:ww], scp[:, 0:ww], Exp)
                    if s2 == 0:
                        # diagonal block: zero the non-causal upper triangle
                        # of the exp weights (0/1 mask on DVE; saves a PE
                        # matmul per key block — each costs ~200ns fixed)
                        nc.vector.tensor_mul(ex[:, 0:P], ex[:, 0:P], czb_sb[:])
                    out.append((ex, w0, ww))
                return out

            def emit_pv(h, kb, exl, at_ps, qcs=range(NQC)):
                # PV pieces per output chunk qc (+denominator via the
                # ones column of v)
                for ex, w0, ww in exl:
                    qc_lo = w0 // SC
                    qc_hi = (w0 + ww - 1) // SC
                    for qc in range(qc_lo, qc_hi + 1):
                        if qc not in qcs:
                            continue
                        a0 = max(w0, qc * SC)
                        a1 = min(w0 + ww, (qc + 1) * SC)
                        nc.tensor.matmul(
                            at_ps[qc][:, a0 - qc * SC : a1 - qc * SC],
                            v_sb[kb][:, h * P : (h + 1) * P],
                            ex[:, a0 - w0 : a1 - w0],
                            start=(kb == 0),
                            stop=(kb == 4 * qc + 3),
                        )

            # ---- phase 3 pieces: own-batch gather + c_proj slices, emitted
            # interleaved into later heads' attention (PE runs in program
            # order). Contraction tile (h, tp): partitions 0:64 = in-batch
            # sender 2tp's head-h rows, 64:128 = sender 2tp+1's.
            osb = {}
            for st in range(4):
                osb[st] = osb_pool.tile([P, NX], f32, name=f"osb{st}")
                # b_proj preloaded across all partitions (idle GpSimd);
                # every proj group is then a plain accumulate
                nc.gpsimd.partition_broadcast(osb[st][:], bp_sb[:])
            lh = {}

            def emit_lh_loads(h):
                for tp in range(2):
                    t = lh_pool.tile([P, SC], bf16, name=f"lh{h}_{tp}")
                    # two half-tile loads on purpose: they land on different
                    # DMA queues and run in parallel (one fused [128,512]
                    # load serializes on a single engine and measures slower)
                    nc.sync.dma_start(t[0:D, :], lhsrc[h][2 * tp])
                    nc.sync.dma_start(t[D:P, :], lhsrc[h][2 * tp + 1])
                    lh[h, tp] = t

            def emit_proj_group(h, st, nn2):
                pp = wide_ps.tile([P, SC], f32, tag="wide", name=f"pj{h}_{st}_{nn2}")
                for tp in range(2):
                    wcol = (2 * h + tp) * NX + nn2 * SC
                    nc.tensor.matmul(
                        pp[:],
                        lh[h, tp][:, st * P : (st + 1) * P],
                        wp_sb[:, wcol : wcol + SC],
                        start=(tp == 0),
                        stop=(tp == 1),
                    )
                dst = osb[st][:, nn2 * SC : (nn2 + 1) * SC]
                nc.vector.tensor_add(dst, dst, pp[:])
                if h == HL - 1 and nn2 == 1:
                    nc.sync.dma_start(out_ext[st * P : (st + 1) * P, :], osb[st][:])

            # ---- dummy A2A fired at the end of phase 1: resyncs the cores'
            # launch/throttle drift right before the real A2As start, and
            # absorbs the collective stream's one-time setup cost ----
            a2ad_in = dram_pool.tile([8, 1, P], bf16, name="a2ad_in")
            a2ad_out = dram_pool.tile([8, 1, P], bf16, name="a2ad_out")
            for s in range(8):
                nc.sync.dma_start(a2ad_in[s], v_sb[15][0:1, 0:P])
            nc.gpsimd.collective_compute(
                "AllToAll",
                mybir.AluOpType.bypass,
                ins=[a2ad_in[:].opt()],
                outs=[a2ad_out[:].opt()],
                replica_groups=[list(range(8))],
            )

            # proj interleave schedule: h0's groups during h2 (late blocks),
            # 4 of h1's groups into h3's tail blocks; h1's other 4 + all of
            # h2's stay after h3 to fill the last A2A's transfer window
            interleave = {2: {}, 3: {}}
            islots = [11, 13, 14, 15]
            for i in range(4):
                interleave[2][islots[i]] = (0, i // 2, i % 2)
            for i in range(4):
                interleave[3][12 + i] = (1, i // 2, i % 2)

            saved = {}
            for h in range(HL):
                at_ps = {}
                done = set()
                for qc in range(NQC):
                    at_ps[qc] = at_ps_pool.tile(
                        [P, SC], f32, tag="atps", name=f"at_ps{qc}_{h}"
                    )
                if h >= 2:
                    emit_lh_loads(h - 2)
                for kb in range(NKB):
                    if (h, kb) in saved:
                        exl = saved.pop((h, kb))
                    else:
                        exl = emit_score_exp(h, kb)
                    # fills between exp and the PV that consumes it: next
                    # head's first (widest) score tiles and earlier heads'
                    # proj groups keep the PE busy while ScalarE runs exp
                    if h + 1 < HL and kb >= NKB - NPRE:
                        kpre = kb - (NKB - NPRE)
                        saved[h + 1, kpre] = emit_score_exp(h + 1, kpre)
                    pj = interleave.get(h, {}).get(kb)
                    if pj is not None:
                        emit_proj_group(*pj)
                    emit_pv(h, kb, exl, at_ps)
                    # immediate normalization: the tail chain is fully
                    # DVE/GpSimd now (no PE broadcast matmul), so deferring
                    # would only delay accumulator frees and A2A staging
                    defer = 0
                    for qc in range(NQC):
                        if kb == min(4 * qc + 3 + defer, NKB - 1) and qc not in done:
                            done.add(qc)
                            emit_tail(h, qc, at_ps[qc])
                nc.gpsimd.collective_compute(
                    "AllToAll",
                    mybir.AluOpType.bypass,
                    ins=[a2a_in[h][:].opt()],
                    outs=[a2a_out[h][:].opt()],
                    replica_groups=[list(range(8))],
                )
                # own-batch slot select: one contiguous 256KB dynamic copy
                nc.sync.dma_start(lhsrc[h][:], a2a_out[h][ds(rv, 4)])

            # tail: h0's and h1's remaining + h2's proj groups overlap the
            # h2/h3 A2A transfers; then the last head's own slice
            for i in range(4):
                emit_proj_group(0, 2 + i // 2, i % 2)
            for i in range(4):
                emit_proj_group(1, 2 + i // 2, i % 2)
            emit_lh_loads(2)
            for st in range(4):
                for nn2 in range(2):
                    emit_proj_group(2, st, nn2)
            emit_lh_loads(3)
            for st in range(4):
                for nn2 in range(2):
                    emit_proj_group(3, st, nn2)

    nc.compile()
    return nc


def _get_compiled():
    global _COMPILED
    if _COMPILED is None:
        _COMPILED = _build()
    return _COMPILED


def make_in_maps(x, attention_mask, w_attn, b_attn, w_proj, b_proj):
    x = np.asarray(x, dtype=np.float32)
    w_attn = np.asarray(w_attn, dtype=np.float32)
    b_attn = np.asarray(b_attn, dtype=np.float32)
    w_proj = np.asarray(w_proj, dtype=np.float32)
    b_proj = np.asarray(b_proj, dtype=np.float32)

    ki, qi = np.meshgrid(np.arange(P), np.arange(P), indexing="ij")
    causalT = np.where(ki > qi, np.float32(0.0), np.float32(1.0))
    # xT [NX, S] -> e-major [NE, P, S], split [:, :, :SC] / [:, :, SC:]
    x8 = [
        np.ascontiguousarray(x[b].T.astype(BF16).reshape(NE, P, S)) for b in range(B)
    ]
    bv_full = b_attn[2 * NX : 3 * NX].astype(np.float64)
    bp_eff = (b_proj.astype(np.float64) + bv_full @ w_proj.astype(np.float64)).astype(
        np.float32
    )
    bp_row32 = np.ascontiguousarray(bp_eff.reshape(1, NX))

    in_maps = []
    for c in range(8):
        b, g = divmod(c, 4)
        cols = slice(HDW * g, HDW * (g + 1))
        kcols = slice(NX + HDW * g, NX + HDW * (g + 1))
        vcols = slice(2 * NX + HDW * g, 2 * NX + HDW * (g + 1))
        bqk_arr = np.concatenate([b_attn[cols] * 0.125, b_attn[kcols]]).reshape(4, P)
        # fi-major q/k weights: wqkf[fi] = [P, NX] with stationary (fi, e)
        # at [:, e*128:(e+1)*128]
        wqk = np.concatenate([w_attn[:, cols], w_attn[:, kcols]], axis=1)  # [NX, 512]
        wqkf = np.ascontiguousarray(
            wqk.reshape(NE, P, 4, P).transpose(2, 1, 0, 3).reshape(4, P, NX)
        ).astype(BF16)
        wvc = np.ascontiguousarray(
            w_attn[:, vcols].reshape(NE, P, HDW).transpose(1, 0, 2).reshape(P, NE * HDW)
        ).astype(BF16)
        # own-batch proj tiles (h, tp): rows 0:64 = in-batch sender 2tp's
        # head-h w_proj rows, 64:128 = sender 2tp+1's
        wtiles = np.zeros((HL, 2, P, NX), dtype=np.float32)
        for h in range(HL):
            for tp in range(2):
                for half, j in ((0, 2 * tp), (1, 2 * tp + 1)):
                    rows = w_proj[HDW * j + D * h : HDW * j + D * (h + 1), :]
                    wtiles[h, tp, 64 * half : 64 * half + D, :] = rows
        wpc = np.ascontiguousarray(
            wtiles.reshape(2 * HL, P, NX).transpose(1, 0, 2).reshape(P, 2 * HL * NX)
        ).astype(BF16)
        in_maps.append(
            {
                "x0": np.ascontiguousarray(x8[b][:, :, :SC]),
                "xr": np.ascontiguousarray(x8[b][:, :, SC:]),
                "wqkf": wqkf,
                "wvc": wvc,
                "wpc": wpc,
                "bqk": np.ascontiguousarray(bqk_arr),
                "bp32": bp_row32,
                "causb": causalT.astype(BF16),
                "onesb": np.ones((P, 4), dtype=BF16),
                "slotb": np.array([[4 * b, 0, 0, 0]], dtype=np.int32),
            }
        )
    return in_maps


def assemble_out(results):
    out = np.empty((B, S, NX), dtype=np.float32)
    for c in range(8):
        b, g = divmod(c, 4)
        out[b, g * SC : (g + 1) * SC, :] = results[c]["out"]
    return out


def run(in_maps, trace=False):
    from concourse.bass_utils import run_bass_kernel_spmd

    nc = _get_compiled()
    return run_bass_kernel_spmd(nc, in_maps, core_ids=list(range(8)), trace=trace)


def kernel(**inputs) -> np.ndarray:
    in_maps = make_in_maps(**inputs)
    res = run(in_maps)
    return assemble_out(res.results)


if __name__ == "__main__":
    _get_compiled()
    print("build+compile OK")

